# revision 12
# baseline (speedup 1.0000x reference)
"""GQA attention (B=2, S=2048, D=4096, 32 Q heads / 8 KV heads, head_dim=128,
RoPE, causal) on 8 Trainium2 NeuronCores, tensor-parallel over heads:
each core owns 4 Q heads + 1 KV head and a column shard of wq/wk/wv plus a
row shard of wo; the wo all-reduce is realized by summing the 8 partial
outputs on the host (the unshard/gather step).

bf16 datapath (inputs pre-cast on host, fp32 PSUM accumulation), Q resident
in SBUF, fine-grained causal diagonal, paired exp calls, fast reciprocal.

Self-contained: all shapes hardcoded; only imports the system toolchain.
"""
import sys
import numpy as np

sys.path.insert(0, '/opt/trn_rl_repo')

import ml_dtypes                       # noqa: E402
import concourse.bass as bass          # noqa: E402
import concourse.mybir as mybir        # noqa: E402
import concourse.tile as tile          # noqa: E402
from concourse import bacc             # noqa: E402
from concourse import bass_utils       # noqa: E402

F32 = mybir.dt.float32
BF16 = mybir.dt.bfloat16
AF = mybir.ActivationFunctionType
NPBF16 = np.dtype(ml_dtypes.bfloat16)

# ---- problem constants ----
N_HEADS = 32
N_KV_HEADS = 8
HEAD_DIM = 128
DIM = 4096
BATCH = 2
SEQ = 2048
N_CORES = 8
HQ = N_HEADS // N_CORES          # q heads per core = 4
SCALE = 1.0 / float(np.sqrt(HEAD_DIM))

_PROGRAM_CACHE = {}


def build_program(batch=BATCH, seq=SEQ):
    """Emit the per-core Bass program (SPMD: identical on all 8 cores)."""
    P = 128
    TW = 512                      # token tile width
    DC = DIM // P                 # 32 contraction chunks
    TB = seq // TW                # t-tiles per batch
    SK = seq // P                 # sk chunks per batch
    DPT = TW // P                 # diagonal chunks per tile = 4

    nc = bacc.Bacc("TRN2", target_bir_lowering=False)

    xT = nc.dram_tensor("xT", [batch, DIM, seq], BF16, kind="ExternalInput").ap()
    wqT = nc.dram_tensor("wqT", [DIM, HQ * P], BF16, kind="ExternalInput").ap()
    wkT = nc.dram_tensor("wkT", [DIM, P], BF16, kind="ExternalInput").ap()
    wvT = nc.dram_tensor("wvT", [DIM, P], BF16, kind="ExternalInput").ap()
    woT = nc.dram_tensor("woT", [HQ * P, DIM], BF16, kind="ExternalInput").ap()
    cosT = nc.dram_tensor("cosT", [64, seq], BF16, kind="ExternalInput").ap()
    sinT = nc.dram_tensor("sinT", [64, seq], BF16, kind="ExternalInput").ap()
    triI = nc.dram_tensor("tri", [P, P], BF16, kind="ExternalInput").ap()
    identI = nc.dram_tensor("ident", [P, P], BF16, kind="ExternalInput").ap()

    OUT = nc.dram_tensor("OUT", [DIM, batch * seq], BF16, kind="ExternalOutput").ap()

    with tile.TileContext(nc) as tc:
        with tc.tile_pool(name="glob", bufs=1) as glob:
            # ---- persistent SBUF state (bf16) ----
            KT_sb = glob.tile([P, batch * seq], BF16)         # [d, tok]
            V_sb = glob.tile([P, batch * SK, P], BF16)        # [t, chunk, d]
            QT_sb = glob.tile([P, HQ, batch * seq], BF16)     # [d, h, tok]
            outT_sb = glob.tile([P, HQ, batch * seq], BF16)   # [d, h, tok]
            cos_sb = glob.tile([64, seq], BF16)
            sin_sb = glob.tile([64, seq], BF16)
            tri_sb = glob.tile([P, P], BF16)
            ident_sb = glob.tile([P, P], BF16)
            ones_sb = glob.tile([P, P], BF16)
            ones_f = glob.tile([P, P], F32)
            wo_sb = glob.tile([P, HQ, DIM], BF16)
            nc.sync.dma_start(cos_sb[:], cosT[:])
            nc.sync.dma_start(sin_sb[:], sinT[:])
            nc.sync.dma_start(tri_sb[:], triI[:])
            nc.sync.dma_start(ident_sb[:], identI[:])
            nc.any.memset(ones_f[:], 1.0)
            nc.vector.tensor_copy(ones_sb[:], ones_f[:])

            # ================= Phase 1: projections + RoPE =================
            with (
                tc.tile_pool(name="wts", bufs=1) as wts,
                tc.tile_pool(name="p1w", bufs=1) as p1w,
                tc.tile_pool(name="ps1", bufs=1, space="PSUM") as ps1,
            ):
                wq_sb = wts.tile([P, DC, HQ * P], BF16)
                wk_sb = wts.tile([P, DC, P], BF16)
                wv_sb = wts.tile([P, DC, P], BF16)
                wqR = wqT.rearrange("(c p) m -> p c m", p=P)
                wkR = wkT.rearrange("(c p) m -> p c m", p=P)
                wvR = wvT.rearrange("(c p) m -> p c m", p=P)

                def rope_write(dst0, dst1, ps, scol):
                    # drain the PSUM bank (casting to bf16) on ACT + DVE in
                    # parallel so the bank frees fast, then rotate on DVE in
                    # bf16 (2x mode)
                    ct = cos_sb[:, scol:scol + TW]
                    st = sin_sb[:, scol:scol + TW]
                    qa = glob.tile([64, TW], BF16, tag="ra", bufs=3, name="qa")
                    qb = glob.tile([64, TW], BF16, tag="rb", bufs=3, name="qb")
                    nc.scalar.copy(qa[:], ps[0:64, :])
                    nc.vector.tensor_copy(qb[:], ps[64:128, :])
                    t0 = glob.tile([64, TW], BF16, tag="rt0", bufs=2, name="t0")
                    t1 = glob.tile([64, TW], BF16, tag="rt1", bufs=2, name="t1")
                    t2 = glob.tile([64, TW], BF16, tag="rt2", bufs=2, name="t2")
                    t3 = glob.tile([64, TW], BF16, tag="rt3", bufs=2, name="t3")
                    nc.vector.tensor_mul(t0[:], qa[:], ct)
                    nc.vector.tensor_mul(t1[:], qb[:], st)
                    nc.vector.tensor_sub(dst0, t0[:], t1[:])
                    nc.vector.tensor_mul(t2[:], qa[:], st)
                    nc.vector.tensor_mul(t3[:], qb[:], ct)
                    nc.vector.tensor_add(dst1, t2[:], t3[:])

                for b in range(batch):
                    for tt in range(TB):
                        scol = tt * TW                 # within-batch col
                        gcol = b * seq + scol          # global col
                        q_ps = []
                        for h in range(HQ):
                            qp = ps1.tile([P, TW], F32, tag=f"q{h}", name=f"qps{h}")
                            q_ps.append(qp)
                        k_ps = ps1.tile([P, TW], F32, tag="k")
                        v_ps = ps1.tile([P, TW], F32, tag="v")
                        for c in range(DC):
                            if b == 0 and tt == 0:
                                nc.sync.dma_start(wq_sb[:, c, :], wqR[:, c, :])
                                nc.sync.dma_start(wk_sb[:, c, :], wkR[:, c, :])
                                nc.sync.dma_start(wv_sb[:, c, :], wvR[:, c, :])
                            xt = p1w.tile([P, TW], BF16, tag="xt", bufs=20)
                            nc.sync.dma_start(
                                xt[:], xT[b, c * P:(c + 1) * P, scol:scol + TW])
                            st = (c == 0)
                            sp = (c == DC - 1)
                            for h in range(HQ):
                                nc.tensor.matmul(
                                    q_ps[h][:], wq_sb[:, c, h * P:(h + 1) * P],
                                    xt[:], start=st, stop=sp)
                            nc.tensor.matmul(k_ps[:], wk_sb[:, c, :], xt[:],
                                             start=st, stop=sp)
                            nc.tensor.matmul(v_ps[:], wv_sb[:, c, :], xt[:],
                                             start=st, stop=sp)

                        # RoPE on Q and K (both resident in SBUF)
                        for h in range(HQ):
                            rope_write(QT_sb[0:64, h, gcol:gcol + TW],
                                       QT_sb[64:128, h, gcol:gcol + TW],
                                       q_ps[h], scol)
                        rope_write(KT_sb[0:64, gcol:gcol + TW],
                                   KT_sb[64:128, gcol:gcol + TW],
                                   k_ps, scol)

                        # V: copy + per-128 transpose to natural layout
                        vtmp = glob.tile([P, TW], BF16, tag="vtmp", bufs=2)
                        nc.scalar.copy(vtmp[:], v_ps[:])
                        for q in range(TW // P):
                            tp = ps1.tile([P, P], BF16, tag="vtp", bufs=2)
                            nc.tensor.transpose(
                                tp[:], vtmp[:, q * P:(q + 1) * P], ident_sb[:])
                            ci = (b * TB + tt) * (TW // P) + q
                            nc.scalar.copy(V_sb[:, ci, :], tp[:])

            # ================= Phase 2: attention =================
            # preload the whole wo shard now: the strided gather is slow and
            # phase-1's DMA window is saturated; it only must land before P3
            woR = woT.rearrange("(g p) m -> p g m", p=P)
            nc.sync.dma_start(wo_sb[:], woR[:])
            with (
                tc.tile_pool(name="p2w", bufs=1) as p2w,
                tc.tile_pool(name="ps2", bufs=1, space="PSUM") as ps2,
            ):
                for b in range(batch):
                    for h in range(HQ):
                        for j in range(TB):
                            gcol = b * seq + j * TW
                            nsk = (j + 1) * DPT
                            den_ps = ps2.tile([P, TW], F32, tag="den", bufs=1)
                            o_ps = ps2.tile([P, TW], F32, tag="opv", bufs=3)
                            # chunk list: (skc, qoff, kw, is_diag)
                            chunks = []
                            for skc in range(nsk):
                                d = skc - DPT * j
                                if d < 0:
                                    chunks.append((skc, 0, TW, False))
                                else:
                                    chunks.append((skc, P * d, TW - P * d, True))
                            # group full-width chunks in pairs for one big exp
                            full = [c for c in chunks if c[2] == TW]
                            rest = [c for c in chunks if c[2] < TW]
                            groups = []
                            i = 0
                            while i + 1 < len(full):
                                groups.append(full[i:i + 2]); i += 2
                            if i < len(full):
                                groups.append([full[i]])
                            for r in rest:
                                groups.append([r])

                            for g in groups:
                                sc2 = ps2.tile([P, 2, TW], F32, tag="sc",
                                               bufs=2, name="sc2")
                                pt = p2w.tile([P, 2, TW], BF16, tag="pt",
                                              bufs=4, name="pt")
                                for gi, (skc, qoff, kw, diag) in enumerate(g):
                                    kcol = b * seq + skc * P
                                    nc.tensor.matmul(
                                        sc2[:, gi, 0:kw],
                                        KT_sb[:, kcol:kcol + P],
                                        QT_sb[:, h, gcol + qoff:gcol + TW],
                                        start=True, stop=True)
                                if len(g) == 2:
                                    nc.scalar.activation(
                                        pt[:, :, :], sc2[:, :, :], AF.Exp,
                                        scale=SCALE)
                                else:
                                    kw = g[0][2]
                                    nc.scalar.activation(
                                        pt[:, 0, 0:kw], sc2[:, 0, 0:kw],
                                        AF.Exp, scale=SCALE)
                                for gi, (skc, qoff, kw, diag) in enumerate(g):
                                    if diag:
                                        nc.vector.tensor_mul(
                                            pt[:, gi, 0:P], pt[:, gi, 0:P],
                                            tri_sb[:])
                                    st = (skc == 0)
                                    sp = (skc == nsk - 1)
                                    # all-ones stationary: every output row
                                    # gets the key-sum -> den pre-broadcast
                                    nc.tensor.matmul(
                                        den_ps[:, qoff:TW], ones_sb[:],
                                        pt[:, gi, 0:kw], start=st, stop=sp,
                                        skip_group_check=True)
                                    ci = b * SK + skc
                                    nc.tensor.matmul(
                                        o_ps[:, qoff:TW], V_sb[:, ci, :],
                                        pt[:, gi, 0:kw], start=st, stop=sp,
                                        skip_group_check=True)

                            bc = p2w.tile([P, TW], F32, tag="bc", bufs=2)
                            nc.vector.reciprocal_approx_fast(bc[:], den_ps[:])
                            nc.vector.tensor_mul(
                                outT_sb[:, h, gcol:gcol + TW], o_ps[:], bc[:])

            # ================= Phase 3: output projection =================
            with (
                tc.tile_pool(name="p3w", bufs=1) as p3w,
                tc.tile_pool(name="ps3", bufs=1, space="PSUM") as ps3,
            ):
                for m in range(DC):
                    for t8 in range(batch * TB):
                        f_ps = ps3.tile([P, TW], F32, tag="f", bufs=4)
                        for h in range(HQ):
                            nc.tensor.matmul(
                                f_ps[:], wo_sb[:, h, m * P:(m + 1) * P],
                                outT_sb[:, h, t8 * TW:(t8 + 1) * TW],
                                start=(h == 0), stop=(h == HQ - 1))
                        og = p3w.tile([P, TW], BF16, tag="og", bufs=4)
                        nc.scalar.copy(og[:], f_ps[:])
                        nc.sync.dma_start(
                            OUT[m * P:(m + 1) * P, t8 * TW:(t8 + 1) * TW],
                            og[:])

    nc.compile()
    return nc


_PERM = np.concatenate([np.arange(0, HEAD_DIM, 2), np.arange(1, HEAD_DIM, 2)])


def prepare_core_inputs(x, freqs_cos, freqs_sin, wq, wk, wv, wo,
                        batch=BATCH, seq=SEQ):
    """Host-side shard + relayout + bf16 cast (pure data movement)."""
    xT = np.ascontiguousarray(
        np.asarray(x, np.float32).transpose(0, 2, 1)).astype(NPBF16)
    cosT = np.ascontiguousarray(np.asarray(freqs_cos, np.float32).T).astype(NPBF16)
    sinT = np.ascontiguousarray(np.asarray(freqs_sin, np.float32).T).astype(NPBF16)
    pp = np.arange(128)[:, None]
    ff = np.arange(128)[None, :]
    tri = (pp <= ff).astype(np.float32).astype(NPBF16)
    ident = np.eye(128, dtype=np.float32).astype(NPBF16)
    wq = np.asarray(wq, np.float32)
    wk = np.asarray(wk, np.float32)
    wv = np.asarray(wv, np.float32)
    wo = np.asarray(wo, np.float32)
    in_maps = []
    for c in range(N_CORES):
        wq_c = wq[c * HQ * HEAD_DIM:(c + 1) * HQ * HEAD_DIM]
        wq_c = wq_c.reshape(HQ, HEAD_DIM, DIM)[:, _PERM, :].reshape(HQ * HEAD_DIM, DIM)
        wk_c = wk[c * HEAD_DIM:(c + 1) * HEAD_DIM][_PERM, :]
        wv_c = wv[c * HEAD_DIM:(c + 1) * HEAD_DIM]
        wo_c = wo[:, c * HQ * HEAD_DIM:(c + 1) * HQ * HEAD_DIM]
        in_maps.append({
            "xT": xT,
            "wqT": np.ascontiguousarray(wq_c.T).astype(NPBF16),
            "wkT": np.ascontiguousarray(wk_c.T).astype(NPBF16),
            "wvT": np.ascontiguousarray(wv_c.T).astype(NPBF16),
            "woT": np.ascontiguousarray(wo_c.T).astype(NPBF16),
            "cosT": cosT,
            "sinT": sinT,
            "tri": tri,
            "ident": ident,
        })
    return in_maps


def run_sharded(in_maps, batch=BATCH, seq=SEQ, trace=False):
    key = (batch, seq)
    if key not in _PROGRAM_CACHE:
        _PROGRAM_CACHE[key] = build_program(batch, seq)
    nc = _PROGRAM_CACHE[key]
    res = bass_utils.run_bass_kernel_spmd(
        nc, in_maps, core_ids=list(range(len(in_maps))), trace=trace)
    return res


def kernel(x, freqs_cos, freqs_sin, wq, wk, wv, wo):
    b, s, _ = np.asarray(x, np.float32).shape
    in_maps = prepare_core_inputs(x, freqs_cos, freqs_sin, wq, wk, wv, wo,
                                  batch=b, seq=s)
    res = run_sharded(in_maps, batch=b, seq=s)
    acc = np.zeros((DIM, b * s), np.float64)
    for r in res.results:
        acc += np.asarray(r["OUT"], np.float64)
    out = acc.astype(np.float32).reshape(DIM, b, s).transpose(1, 2, 0)
    return np.ascontiguousarray(out)


# revision 16
# speedup vs baseline: 1.1837x; 1.1837x over previous
"""GQA attention (B=2, S=2048, D=4096, 32 Q heads / 8 KV heads, head_dim=128,
RoPE, causal) on 8 Trainium2 NeuronCores, tensor-parallel over heads:
each core owns 4 Q heads + 1 KV head and a column shard of wq/wk/wv plus a
row shard of wo; the wo all-reduce is realized by summing the 8 partial
outputs on the host (the unshard/gather step).

bf16 datapath (inputs pre-cast on host, fp32 PSUM accumulation), Q resident
in SBUF, fine-grained causal diagonal, paired exp calls, fast reciprocal.

Self-contained: all shapes hardcoded; only imports the system toolchain.
"""
import sys
import numpy as np

sys.path.insert(0, '/opt/trn_rl_repo')

import ml_dtypes                       # noqa: E402
import concourse.bass as bass          # noqa: E402
import concourse.mybir as mybir        # noqa: E402
import concourse.tile as tile          # noqa: E402
from concourse import bacc             # noqa: E402
from concourse import bass_utils       # noqa: E402

F32 = mybir.dt.float32
BF16 = mybir.dt.bfloat16
AF = mybir.ActivationFunctionType
NPBF16 = np.dtype(ml_dtypes.bfloat16)

# ---- problem constants ----
N_HEADS = 32
N_KV_HEADS = 8
HEAD_DIM = 128
DIM = 4096
BATCH = 2
SEQ = 2048
N_CORES = 8
HQ = N_HEADS // N_CORES          # q heads per core = 4
SCALE = 1.0 / float(np.sqrt(HEAD_DIM))

_PROGRAM_CACHE = {}


def build_program(batch=BATCH, seq=SEQ):
    """Emit the per-core Bass program (SPMD: identical on all 8 cores)."""
    P = 128
    TW = 512                      # token tile width
    DC = DIM // P                 # 32 contraction chunks
    TB = seq // TW                # t-tiles per batch
    SK = seq // P                 # sk chunks per batch
    DPT = TW // P                 # diagonal chunks per tile = 4

    nc = bacc.Bacc("TRN2", target_bir_lowering=False)

    xT = nc.dram_tensor("xT", [batch, DIM, seq], BF16, kind="ExternalInput").ap()
    wqT = nc.dram_tensor("wqT", [DIM, HQ * P], BF16, kind="ExternalInput").ap()
    wkT = nc.dram_tensor("wkT", [DIM, P], BF16, kind="ExternalInput").ap()
    wvT = nc.dram_tensor("wvT", [DIM, P], BF16, kind="ExternalInput").ap()
    woT = nc.dram_tensor("woT", [HQ * P, DIM], BF16, kind="ExternalInput").ap()
    cosT = nc.dram_tensor("cosT", [64, seq], BF16, kind="ExternalInput").ap()
    sinT = nc.dram_tensor("sinT", [64, seq], BF16, kind="ExternalInput").ap()
    triI = nc.dram_tensor("tri", [P, P], BF16, kind="ExternalInput").ap()
    identI = nc.dram_tensor("ident", [P, P], BF16, kind="ExternalInput").ap()

    OUT = nc.dram_tensor("OUT", [DIM, batch * seq], BF16, kind="ExternalOutput").ap()

    with tile.TileContext(nc) as tc:
        with tc.tile_pool(name="glob", bufs=1) as glob:
            # ---- persistent SBUF state (bf16) ----
            KT_sb = glob.tile([P, batch * seq], BF16)         # [d, tok]
            V_sb = glob.tile([P, batch * SK, P], BF16)        # [t, chunk, d]
            QT_sb = glob.tile([P, HQ, batch * seq], BF16)     # [d, h, tok]
            outT_sb = glob.tile([P, HQ, batch * seq], BF16)   # [d, h, tok]
            cos_sb = glob.tile([64, seq], BF16)
            sin_sb = glob.tile([64, seq], BF16)
            tri_sb = glob.tile([P, P], BF16)
            ident_sb = glob.tile([P, P], BF16)
            ones_sb = glob.tile([P, P], BF16)
            ones_f = glob.tile([P, P], F32)
            wo_sb = glob.tile([P, HQ, DIM], BF16)
            nc.sync.dma_start(cos_sb[:], cosT[:])
            nc.sync.dma_start(sin_sb[:], sinT[:])
            nc.sync.dma_start(tri_sb[:], triI[:])
            nc.sync.dma_start(ident_sb[:], identI[:])
            nc.any.memset(ones_f[:], 1.0)
            nc.vector.tensor_copy(ones_sb[:], ones_f[:])

            # ================= Phase 1: projections + RoPE =================
            with (
                tc.tile_pool(name="wts", bufs=1) as wts,
                tc.tile_pool(name="p1w", bufs=1) as p1w,
                tc.tile_pool(name="ps1", bufs=1, space="PSUM") as ps1,
            ):
                wq_sb = wts.tile([P, DC, HQ * P], BF16)
                wk_sb = wts.tile([P, DC, P], BF16)
                wv_sb = wts.tile([P, DC, P], BF16)
                wqR = wqT.rearrange("(c p) m -> p c m", p=P)
                wkR = wkT.rearrange("(c p) m -> p c m", p=P)
                wvR = wvT.rearrange("(c p) m -> p c m", p=P)

                def rope_write(dst0, dst1, ps, scol):
                    # drain the PSUM bank (casting to bf16) on ACT + DVE in
                    # parallel so the bank frees fast, then rotate on DVE in
                    # bf16 (2x mode)
                    ct = cos_sb[:, scol:scol + TW]
                    st = sin_sb[:, scol:scol + TW]
                    qa = glob.tile([64, TW], BF16, tag="ra", bufs=3, name="qa")
                    qb = glob.tile([64, TW], BF16, tag="rb", bufs=3, name="qb")
                    nc.scalar.copy(qa[:], ps[0:64, :])
                    nc.vector.tensor_copy(qb[:], ps[64:128, :])
                    t0 = glob.tile([64, TW], BF16, tag="rt0", bufs=2, name="t0")
                    t1 = glob.tile([64, TW], BF16, tag="rt1", bufs=2, name="t1")
                    t2 = glob.tile([64, TW], BF16, tag="rt2", bufs=2, name="t2")
                    t3 = glob.tile([64, TW], BF16, tag="rt3", bufs=2, name="t3")
                    nc.vector.tensor_mul(t0[:], qa[:], ct)
                    nc.vector.tensor_mul(t1[:], qb[:], st)
                    nc.vector.tensor_sub(dst0, t0[:], t1[:])
                    nc.vector.tensor_mul(t2[:], qa[:], st)
                    nc.vector.tensor_mul(t3[:], qb[:], ct)
                    nc.vector.tensor_add(dst1, t2[:], t3[:])

                for b in range(batch):
                    for tt in range(TB):
                        scol = tt * TW                 # within-batch col
                        gcol = b * seq + scol          # global col
                        q_ps = []
                        for h in range(HQ):
                            qp = ps1.tile([P, TW], F32, tag=f"q{h}", name=f"qps{h}")
                            q_ps.append(qp)
                        k_ps = ps1.tile([P, TW], F32, tag="k")
                        v_ps = ps1.tile([P, TW], F32, tag="v")
                        for c in range(DC):
                            if b == 0 and tt == 0:
                                nc.sync.dma_start(wq_sb[:, c, :], wqR[:, c, :])
                                nc.sync.dma_start(wk_sb[:, c, :], wkR[:, c, :])
                                nc.sync.dma_start(wv_sb[:, c, :], wvR[:, c, :])
                            xt = p1w.tile([P, TW], BF16, tag="xt", bufs=12)
                            nc.sync.dma_start(
                                xt[:], xT[b, c * P:(c + 1) * P, scol:scol + TW])
                            st = (c == 0)
                            sp = (c == DC - 1)
                            for h in range(HQ):
                                nc.tensor.matmul(
                                    q_ps[h][:], wq_sb[:, c, h * P:(h + 1) * P],
                                    xt[:], start=st, stop=sp)
                            nc.tensor.matmul(k_ps[:], wk_sb[:, c, :], xt[:],
                                             start=st, stop=sp)
                            nc.tensor.matmul(v_ps[:], wv_sb[:, c, :], xt[:],
                                             start=st, stop=sp)

                        # RoPE on Q and K (both resident in SBUF)
                        for h in range(HQ):
                            rope_write(QT_sb[0:64, h, gcol:gcol + TW],
                                       QT_sb[64:128, h, gcol:gcol + TW],
                                       q_ps[h], scol)
                        rope_write(KT_sb[0:64, gcol:gcol + TW],
                                   KT_sb[64:128, gcol:gcol + TW],
                                   k_ps, scol)

                        # V: copy + per-128 transpose to natural layout
                        vtmp = glob.tile([P, TW], BF16, tag="vtmp", bufs=2)
                        nc.scalar.copy(vtmp[:], v_ps[:])
                        for q in range(TW // P):
                            tp = ps1.tile([P, P], BF16, tag="vtp", bufs=2)
                            nc.tensor.transpose(
                                tp[:], vtmp[:, q * P:(q + 1) * P], ident_sb[:])
                            ci = (b * TB + tt) * (TW // P) + q
                            nc.scalar.copy(V_sb[:, ci, :], tp[:])

            # ================= Phase 2: attention =================
            # preload the whole wo shard now: the strided gather is slow and
            # phase-1's DMA window is saturated; it only must land before P3
            woR = woT.rearrange("(g p) m -> p g m", p=P)
            nc.sync.dma_start(wo_sb[:], woR[:])
            with (
                tc.tile_pool(name="p2w", bufs=1) as p2w,
                tc.tile_pool(name="ps2", bufs=1, space="PSUM") as ps2,
            ):
                for b in range(batch):
                    for h in range(HQ):
                        for j in range(TB):
                            gcol = b * seq + j * TW
                            nsk = (j + 1) * DPT
                            den_ps = ps2.tile([P, TW], F32, tag="den", bufs=2)
                            o_ps = ps2.tile([P, TW], F32, tag="opv", bufs=2)
                            # chunk list: (skc, qoff, kw, is_diag)
                            chunks = []
                            for skc in range(nsk):
                                d = skc - DPT * j
                                if d < 0:
                                    chunks.append((skc, 0, TW, False))
                                else:
                                    chunks.append((skc, P * d, TW - P * d, True))
                            # group full-width chunks in pairs for one big exp;
                            # narrow diagonal groups go FIRST so the iteration
                            # tail is full-width PE-bound work (any exec order
                            # is correct: the first emitted matmul's start=True
                            # clears the whole bank's has_written bits)
                            full = [c for c in chunks if c[2] == TW]
                            rest = [c for c in chunks if c[2] < TW]
                            groups = [[r] for r in rest]
                            i = 0
                            while i + 1 < len(full):
                                groups.append(full[i:i + 2]); i += 2
                            if i < len(full):
                                groups.append([full[i]])

                            first_mm = True
                            last_group = groups[-1]
                            for g in groups:
                                sc2 = ps2.tile([P, 2, TW], F32, tag="sc",
                                               bufs=2, name="sc2")
                                pt = p2w.tile([P, 2, TW], BF16, tag="pt",
                                              bufs=4, name="pt")
                                for gi, (skc, qoff, kw, diag) in enumerate(g):
                                    kcol = b * seq + skc * P
                                    nc.tensor.matmul(
                                        sc2[:, gi, 0:kw],
                                        KT_sb[:, kcol:kcol + P],
                                        QT_sb[:, h, gcol + qoff:gcol + TW],
                                        start=True, stop=True)
                                if len(g) == 2:
                                    nc.scalar.activation(
                                        pt[:, :, :], sc2[:, :, :], AF.Exp,
                                        scale=SCALE)
                                else:
                                    kw = g[0][2]
                                    nc.scalar.activation(
                                        pt[:, 0, 0:kw], sc2[:, 0, 0:kw],
                                        AF.Exp, scale=SCALE)
                                for gi, (skc, qoff, kw, diag) in enumerate(g):
                                    if diag:
                                        nc.vector.tensor_mul(
                                            pt[:, gi, 0:P], pt[:, gi, 0:P],
                                            tri_sb[:])
                                    st = first_mm
                                    first_mm = False
                                    sp = (g is last_group) and (gi == len(g) - 1)
                                    # all-ones stationary: every output row
                                    # gets the key-sum -> den pre-broadcast
                                    nc.tensor.matmul(
                                        den_ps[:, qoff:TW], ones_sb[:],
                                        pt[:, gi, 0:kw], start=st, stop=sp,
                                        skip_group_check=True)
                                    ci = b * SK + skc
                                    nc.tensor.matmul(
                                        o_ps[:, qoff:TW], V_sb[:, ci, :],
                                        pt[:, gi, 0:kw], start=st, stop=sp,
                                        skip_group_check=True)

                            bc = p2w.tile([P, TW], F32, tag="bc", bufs=2)
                            nc.vector.reciprocal_approx_fast(bc[:], den_ps[:])
                            nc.vector.tensor_mul(
                                outT_sb[:, h, gcol:gcol + TW], o_ps[:], bc[:])

            # ================= Phase 3: output projection =================
            with (
                tc.tile_pool(name="p3w", bufs=1) as p3w,
                tc.tile_pool(name="ps3", bufs=1, space="PSUM") as ps3,
            ):
                for m in range(DC):
                    for t8 in range(batch * TB):
                        f_ps = ps3.tile([P, TW], F32, tag="f", bufs=4)
                        for h in range(HQ):
                            nc.tensor.matmul(
                                f_ps[:], wo_sb[:, h, m * P:(m + 1) * P],
                                outT_sb[:, h, t8 * TW:(t8 + 1) * TW],
                                start=(h == 0), stop=(h == HQ - 1))
                        og = p3w.tile([P, TW], BF16, tag="og", bufs=4)
                        nc.scalar.copy(og[:], f_ps[:])
                        nc.sync.dma_start(
                            OUT[m * P:(m + 1) * P, t8 * TW:(t8 + 1) * TW],
                            og[:])

    nc.compile()
    return nc


_PERM = np.concatenate([np.arange(0, HEAD_DIM, 2), np.arange(1, HEAD_DIM, 2)])


def prepare_core_inputs(x, freqs_cos, freqs_sin, wq, wk, wv, wo,
                        batch=BATCH, seq=SEQ):
    """Host-side shard + relayout + bf16 cast (pure data movement)."""
    xT = np.ascontiguousarray(
        np.asarray(x, np.float32).transpose(0, 2, 1)).astype(NPBF16)
    cosT = np.ascontiguousarray(np.asarray(freqs_cos, np.float32).T).astype(NPBF16)
    sinT = np.ascontiguousarray(np.asarray(freqs_sin, np.float32).T).astype(NPBF16)
    pp = np.arange(128)[:, None]
    ff = np.arange(128)[None, :]
    tri = (pp <= ff).astype(np.float32).astype(NPBF16)
    ident = np.eye(128, dtype=np.float32).astype(NPBF16)
    wq = np.asarray(wq, np.float32)
    wk = np.asarray(wk, np.float32)
    wv = np.asarray(wv, np.float32)
    wo = np.asarray(wo, np.float32)
    in_maps = []
    for c in range(N_CORES):
        wq_c = wq[c * HQ * HEAD_DIM:(c + 1) * HQ * HEAD_DIM]
        wq_c = wq_c.reshape(HQ, HEAD_DIM, DIM)[:, _PERM, :].reshape(HQ * HEAD_DIM, DIM)
        wk_c = wk[c * HEAD_DIM:(c + 1) * HEAD_DIM][_PERM, :]
        wv_c = wv[c * HEAD_DIM:(c + 1) * HEAD_DIM]
        wo_c = wo[:, c * HQ * HEAD_DIM:(c + 1) * HQ * HEAD_DIM]
        in_maps.append({
            "xT": xT,
            "wqT": np.ascontiguousarray(wq_c.T).astype(NPBF16),
            "wkT": np.ascontiguousarray(wk_c.T).astype(NPBF16),
            "wvT": np.ascontiguousarray(wv_c.T).astype(NPBF16),
            "woT": np.ascontiguousarray(wo_c.T).astype(NPBF16),
            "cosT": cosT,
            "sinT": sinT,
            "tri": tri,
            "ident": ident,
        })
    return in_maps


def run_sharded(in_maps, batch=BATCH, seq=SEQ, trace=False):
    key = (batch, seq)
    if key not in _PROGRAM_CACHE:
        _PROGRAM_CACHE[key] = build_program(batch, seq)
    nc = _PROGRAM_CACHE[key]
    res = bass_utils.run_bass_kernel_spmd(
        nc, in_maps, core_ids=list(range(len(in_maps))), trace=trace)
    return res


def kernel(x, freqs_cos, freqs_sin, wq, wk, wv, wo):
    b, s, _ = np.asarray(x, np.float32).shape
    in_maps = prepare_core_inputs(x, freqs_cos, freqs_sin, wq, wk, wv, wo,
                                  batch=b, seq=s)
    res = run_sharded(in_maps, batch=b, seq=s)
    acc = np.zeros((DIM, b * s), np.float64)
    for r in res.results:
        acc += np.asarray(r["OUT"], np.float64)
    out = acc.astype(np.float32).reshape(DIM, b, s).transpose(1, 2, 0)
    return np.ascontiguousarray(out)


# revision 20
# speedup vs baseline: 1.1962x; 1.0106x over previous
"""GQA attention (B=2, S=2048, D=4096, 32 Q heads / 8 KV heads, head_dim=128,
RoPE, causal) on 8 Trainium2 NeuronCores, tensor-parallel over heads:
each core owns 4 Q heads + 1 KV head and a column shard of wq/wk/wv plus a
row shard of wo; the wo all-reduce is realized by summing the 8 partial
outputs on the host (the unshard/gather step).

bf16 datapath (inputs pre-cast on host, fp32 PSUM accumulation), Q resident
in SBUF, fine-grained causal diagonal, paired exp calls, fast reciprocal.

Self-contained: all shapes hardcoded; only imports the system toolchain.
"""
import sys
import numpy as np

sys.path.insert(0, '/opt/trn_rl_repo')

import ml_dtypes                       # noqa: E402
import concourse.bass as bass          # noqa: E402
import concourse.mybir as mybir        # noqa: E402
import concourse.tile as tile          # noqa: E402
from concourse import bacc             # noqa: E402
from concourse import bass_utils       # noqa: E402

F32 = mybir.dt.float32
BF16 = mybir.dt.bfloat16
AF = mybir.ActivationFunctionType
NPBF16 = np.dtype(ml_dtypes.bfloat16)

# ---- problem constants ----
N_HEADS = 32
N_KV_HEADS = 8
HEAD_DIM = 128
DIM = 4096
BATCH = 2
SEQ = 2048
N_CORES = 8
HQ = N_HEADS // N_CORES          # q heads per core = 4
SCALE = 1.0 / float(np.sqrt(HEAD_DIM))

_PROGRAM_CACHE = {}


def build_program(batch=BATCH, seq=SEQ):
    """Emit the per-core Bass program (SPMD: identical on all 8 cores)."""
    P = 128
    TW = 512                      # token tile width
    DC = DIM // P                 # 32 contraction chunks
    TB = seq // TW                # t-tiles per batch
    SK = seq // P                 # sk chunks per batch
    DPT = TW // P                 # diagonal chunks per tile = 4

    nc = bacc.Bacc("TRN2", target_bir_lowering=False)

    xT = nc.dram_tensor("xT", [batch, DIM, seq], BF16, kind="ExternalInput").ap()
    wqT = nc.dram_tensor("wqT", [DIM, HQ * P], BF16, kind="ExternalInput").ap()
    wkT = nc.dram_tensor("wkT", [DIM, P], BF16, kind="ExternalInput").ap()
    wvT = nc.dram_tensor("wvT", [DIM, P], BF16, kind="ExternalInput").ap()
    woT = nc.dram_tensor("woT", [HQ * P, DIM], BF16, kind="ExternalInput").ap()
    cosT = nc.dram_tensor("cosT", [64, seq], BF16, kind="ExternalInput").ap()
    sinT = nc.dram_tensor("sinT", [64, seq], BF16, kind="ExternalInput").ap()
    triI = nc.dram_tensor("tri", [P, P], BF16, kind="ExternalInput").ap()
    identI = nc.dram_tensor("ident", [P, P], BF16, kind="ExternalInput").ap()

    OUT = nc.dram_tensor("OUT", [DIM, batch * seq], BF16, kind="ExternalOutput").ap()

    with tile.TileContext(nc) as tc:
        with tc.tile_pool(name="glob", bufs=1) as glob:
            # ---- persistent SBUF state (bf16) ----
            KT_sb = glob.tile([P, batch * seq], BF16)         # [d, tok]
            V_sb = glob.tile([P, batch * SK, P], BF16)        # [t, chunk, d]
            QT_sb = glob.tile([P, HQ, batch * seq], BF16)     # [d, h, tok]
            outT_sb = glob.tile([P, HQ, batch * seq], BF16)   # [d, h, tok]
            cos_sb = glob.tile([64, seq], BF16)
            sin_sb = glob.tile([64, seq], BF16)
            tri_sb = glob.tile([P, P], BF16)
            ident_sb = glob.tile([P, P], BF16)
            ones_sb = glob.tile([P, P], BF16)
            ones_f = glob.tile([P, P], F32)
            wo_sb = glob.tile([P, HQ, DIM], BF16)
            nc.sync.dma_start(cos_sb[:], cosT[:])
            nc.sync.dma_start(sin_sb[:], sinT[:])
            nc.sync.dma_start(tri_sb[:], triI[:])
            nc.sync.dma_start(ident_sb[:], identI[:])
            nc.any.memset(ones_f[:], 1.0)
            nc.vector.tensor_copy(ones_sb[:], ones_f[:])

            # ================= Phase 1: projections + RoPE =================
            with (
                tc.tile_pool(name="wts", bufs=1) as wts,
                tc.tile_pool(name="p1w", bufs=1) as p1w,
                tc.tile_pool(name="ps1", bufs=1, space="PSUM") as ps1,
            ):
                wq_sb = wts.tile([P, DC, HQ * P], BF16)
                wk_sb = wts.tile([P, DC, P], BF16)
                wv_sb = wts.tile([P, DC, P], BF16)
                wqR = wqT.rearrange("(c p) m -> p c m", p=P)
                wkR = wkT.rearrange("(c p) m -> p c m", p=P)
                wvR = wvT.rearrange("(c p) m -> p c m", p=P)

                def rope_write(dst0, dst1, sf, scol):
                    # rotate on DVE in bf16 (2x mode) from an SBUF staging
                    # tile (PSUM already drained by a single full-bank copy);
                    # the odd half gets rebased to partition 0 with a cheap
                    # single-input bf16 copy (TT needs equal input bases)
                    ct = cos_sb[:, scol:scol + TW]
                    st = sin_sb[:, scol:scol + TW]
                    qa = sf[0:64, :]
                    qbh = glob.tile([64, TW], BF16, tag="qbh", bufs=3, name="qbh")
                    nc.vector.tensor_copy(qbh[:], sf[64:128, :])
                    qb = qbh[:]
                    t0 = glob.tile([64, TW], BF16, tag="rt0", bufs=2, name="t0")
                    t1 = glob.tile([64, TW], BF16, tag="rt1", bufs=2, name="t1")
                    t2 = glob.tile([64, TW], BF16, tag="rt2", bufs=2, name="t2")
                    t3 = glob.tile([64, TW], BF16, tag="rt3", bufs=2, name="t3")
                    nc.vector.tensor_mul(t0[:], qa, ct)
                    nc.vector.tensor_mul(t1[:], qb, st)
                    nc.vector.tensor_sub(dst0, t0[:], t1[:])
                    nc.vector.tensor_mul(t2[:], qa, st)
                    nc.vector.tensor_mul(t3[:], qb, ct)
                    nc.vector.tensor_add(dst1, t2[:], t3[:])

                for b in range(batch):
                    for tt in range(TB):
                        scol = tt * TW                 # within-batch col
                        gcol = b * seq + scol          # global col
                        q_ps = []
                        for h in range(HQ):
                            qp = ps1.tile([P, TW], F32, tag=f"q{h}", name=f"qps{h}")
                            q_ps.append(qp)
                        k_ps = ps1.tile([P, TW], F32, tag="k")
                        v_ps = ps1.tile([P, TW], F32, tag="v")
                        for c in range(DC):
                            if b == 0 and tt == 0:
                                nc.sync.dma_start(wq_sb[:, c, :], wqR[:, c, :])
                                nc.sync.dma_start(wk_sb[:, c, :], wkR[:, c, :])
                                nc.sync.dma_start(wv_sb[:, c, :], wvR[:, c, :])
                            xt = p1w.tile([P, TW], BF16, tag="xt", bufs=12)
                            nc.sync.dma_start(
                                xt[:], xT[b, c * P:(c + 1) * P, scol:scol + TW])
                            st = (c == 0)
                            sp = (c == DC - 1)
                            for h in range(HQ):
                                nc.tensor.matmul(
                                    q_ps[h][:], wq_sb[:, c, h * P:(h + 1) * P],
                                    xt[:], start=st, stop=sp)
                            nc.tensor.matmul(k_ps[:], wk_sb[:, c, :], xt[:],
                                             start=st, stop=sp)
                            nc.tensor.matmul(v_ps[:], wv_sb[:, c, :], xt[:],
                                             start=st, stop=sp)

                        # drain all 6 PSUM banks fast: one full-bank bf16
                        # copy each, alternating ACT / DVE
                        stg = []
                        for h in range(HQ):
                            s = glob.tile([P, TW], BF16, tag="stage", bufs=8,
                                          name=f"stg{h}")
                            if h % 2 == 0:
                                nc.scalar.copy(s[:], q_ps[h][:])
                            else:
                                nc.vector.tensor_copy(s[:], q_ps[h][:])
                            stg.append(s)
                        skf = glob.tile([P, TW], BF16, tag="stage", bufs=8,
                                        name="stgk")
                        nc.scalar.copy(skf[:], k_ps[:])
                        vtmp = glob.tile([P, TW], BF16, tag="stage", bufs=8,
                                         name="stgv")
                        nc.vector.tensor_copy(vtmp[:], v_ps[:])

                        # RoPE on Q and K (both resident in SBUF)
                        for h in range(HQ):
                            rope_write(QT_sb[0:64, h, gcol:gcol + TW],
                                       QT_sb[64:128, h, gcol:gcol + TW],
                                       stg[h], scol)
                        rope_write(KT_sb[0:64, gcol:gcol + TW],
                                   KT_sb[64:128, gcol:gcol + TW],
                                   skf, scol)

                        # V: per-128 transpose to natural layout
                        for q in range(TW // P):
                            tp = ps1.tile([P, P], BF16, tag="vtp", bufs=2)
                            nc.tensor.transpose(
                                tp[:], vtmp[:, q * P:(q + 1) * P], ident_sb[:])
                            ci = (b * TB + tt) * (TW // P) + q
                            nc.scalar.copy(V_sb[:, ci, :], tp[:])

            # ================= Phase 2: attention =================
            # preload the whole wo shard now: the strided gather is slow and
            # phase-1's DMA window is saturated; it only must land before P3
            woR = woT.rearrange("(g p) m -> p g m", p=P)
            nc.sync.dma_start(wo_sb[:], woR[:])
            with (
                tc.tile_pool(name="p2w", bufs=1) as p2w,
                tc.tile_pool(name="ps2", bufs=1, space="PSUM") as ps2,
            ):
                for b in range(batch):
                    for h in range(HQ):
                        for j in range(TB):
                            gcol = b * seq + j * TW
                            nsk = (j + 1) * DPT
                            den_ps = ps2.tile([P, TW], F32, tag="den", bufs=2)
                            o_ps = ps2.tile([P, TW], F32, tag="opv", bufs=2)
                            # chunk list: (skc, qoff, kw, is_diag)
                            chunks = []
                            for skc in range(nsk):
                                d = skc - DPT * j
                                if d < 0:
                                    chunks.append((skc, 0, TW, False))
                                else:
                                    chunks.append((skc, P * d, TW - P * d, True))
                            # group full-width chunks in pairs for one big exp;
                            # narrow diagonal groups go FIRST so the iteration
                            # tail is full-width PE-bound work (any exec order
                            # is correct: the first emitted matmul's start=True
                            # clears the whole bank's has_written bits)
                            full = [c for c in chunks if c[2] == TW]
                            rest = [c for c in chunks if c[2] < TW]
                            groups = [[r] for r in rest]
                            i = 0
                            while i + 1 < len(full):
                                groups.append(full[i:i + 2]); i += 2
                            if i < len(full):
                                groups.append([full[i]])

                            first_mm = True
                            last_group = groups[-1]
                            for g in groups:
                                sc2 = ps2.tile([P, 2, TW], F32, tag="sc",
                                               bufs=2, name="sc2")
                                pt = p2w.tile([P, 2, TW], BF16, tag="pt",
                                              bufs=6, name="pt")
                                for gi, (skc, qoff, kw, diag) in enumerate(g):
                                    kcol = b * seq + skc * P
                                    nc.tensor.matmul(
                                        sc2[:, gi, 0:kw],
                                        KT_sb[:, kcol:kcol + P],
                                        QT_sb[:, h, gcol + qoff:gcol + TW],
                                        start=True, stop=True)
                                if len(g) == 2:
                                    nc.scalar.activation(
                                        pt[:, :, :], sc2[:, :, :], AF.Exp,
                                        scale=SCALE)
                                else:
                                    kw = g[0][2]
                                    nc.scalar.activation(
                                        pt[:, 0, 0:kw], sc2[:, 0, 0:kw],
                                        AF.Exp, scale=SCALE)
                                for gi, (skc, qoff, kw, diag) in enumerate(g):
                                    if diag:
                                        nc.vector.tensor_mul(
                                            pt[:, gi, 0:P], pt[:, gi, 0:P],
                                            tri_sb[:])
                                    st = first_mm
                                    first_mm = False
                                    sp = (g is last_group) and (gi == len(g) - 1)
                                    # all-ones stationary: every output row
                                    # gets the key-sum -> den pre-broadcast
                                    nc.tensor.matmul(
                                        den_ps[:, qoff:TW], ones_sb[:],
                                        pt[:, gi, 0:kw], start=st, stop=sp,
                                        skip_group_check=True)
                                    ci = b * SK + skc
                                    nc.tensor.matmul(
                                        o_ps[:, qoff:TW], V_sb[:, ci, :],
                                        pt[:, gi, 0:kw], start=st, stop=sp,
                                        skip_group_check=True)

                            bc = p2w.tile([P, TW], F32, tag="bc", bufs=2)
                            nc.vector.reciprocal_approx_fast(bc[:], den_ps[:])
                            nc.vector.tensor_mul(
                                outT_sb[:, h, gcol:gcol + TW], o_ps[:], bc[:])

            # ================= Phase 3: output projection =================
            with (
                tc.tile_pool(name="p3w", bufs=1) as p3w,
                tc.tile_pool(name="ps3", bufs=1, space="PSUM") as ps3,
            ):
                for m in range(DC):
                    for t8 in range(batch * TB):
                        f_ps = ps3.tile([P, TW], F32, tag="f", bufs=4)
                        for h in range(HQ):
                            nc.tensor.matmul(
                                f_ps[:], wo_sb[:, h, m * P:(m + 1) * P],
                                outT_sb[:, h, t8 * TW:(t8 + 1) * TW],
                                start=(h == 0), stop=(h == HQ - 1))
                        og = p3w.tile([P, TW], BF16, tag="og", bufs=4)
                        nc.scalar.copy(og[:], f_ps[:])
                        nc.sync.dma_start(
                            OUT[m * P:(m + 1) * P, t8 * TW:(t8 + 1) * TW],
                            og[:])

    nc.compile()
    return nc


_PERM = np.concatenate([np.arange(0, HEAD_DIM, 2), np.arange(1, HEAD_DIM, 2)])


def prepare_core_inputs(x, freqs_cos, freqs_sin, wq, wk, wv, wo,
                        batch=BATCH, seq=SEQ):
    """Host-side shard + relayout + bf16 cast (pure data movement)."""
    xT = np.ascontiguousarray(
        np.asarray(x, np.float32).transpose(0, 2, 1)).astype(NPBF16)
    cosT = np.ascontiguousarray(np.asarray(freqs_cos, np.float32).T).astype(NPBF16)
    sinT = np.ascontiguousarray(np.asarray(freqs_sin, np.float32).T).astype(NPBF16)
    pp = np.arange(128)[:, None]
    ff = np.arange(128)[None, :]
    tri = (pp <= ff).astype(np.float32).astype(NPBF16)
    ident = np.eye(128, dtype=np.float32).astype(NPBF16)
    wq = np.asarray(wq, np.float32)
    wk = np.asarray(wk, np.float32)
    wv = np.asarray(wv, np.float32)
    wo = np.asarray(wo, np.float32)
    in_maps = []
    for c in range(N_CORES):
        wq_c = wq[c * HQ * HEAD_DIM:(c + 1) * HQ * HEAD_DIM]
        wq_c = wq_c.reshape(HQ, HEAD_DIM, DIM)[:, _PERM, :].reshape(HQ * HEAD_DIM, DIM)
        wk_c = wk[c * HEAD_DIM:(c + 1) * HEAD_DIM][_PERM, :]
        wv_c = wv[c * HEAD_DIM:(c + 1) * HEAD_DIM]
        wo_c = wo[:, c * HQ * HEAD_DIM:(c + 1) * HQ * HEAD_DIM]
        in_maps.append({
            "xT": xT,
            "wqT": np.ascontiguousarray(wq_c.T).astype(NPBF16),
            "wkT": np.ascontiguousarray(wk_c.T).astype(NPBF16),
            "wvT": np.ascontiguousarray(wv_c.T).astype(NPBF16),
            "woT": np.ascontiguousarray(wo_c.T).astype(NPBF16),
            "cosT": cosT,
            "sinT": sinT,
            "tri": tri,
            "ident": ident,
        })
    return in_maps


def run_sharded(in_maps, batch=BATCH, seq=SEQ, trace=False):
    key = (batch, seq)
    if key not in _PROGRAM_CACHE:
        _PROGRAM_CACHE[key] = build_program(batch, seq)
    nc = _PROGRAM_CACHE[key]
    res = bass_utils.run_bass_kernel_spmd(
        nc, in_maps, core_ids=list(range(len(in_maps))), trace=trace)
    return res


def kernel(x, freqs_cos, freqs_sin, wq, wk, wv, wo):
    b, s, _ = np.asarray(x, np.float32).shape
    in_maps = prepare_core_inputs(x, freqs_cos, freqs_sin, wq, wk, wv, wo,
                                  batch=b, seq=s)
    res = run_sharded(in_maps, batch=b, seq=s)
    acc = np.zeros((DIM, b * s), np.float64)
    for r in res.results:
        acc += np.asarray(r["OUT"], np.float64)
    out = acc.astype(np.float32).reshape(DIM, b, s).transpose(1, 2, 0)
    return np.ascontiguousarray(out)
